# revision 17
# baseline (speedup 1.0000x reference)
"""Trainium2 Bass kernel for MultiHeadedAttention with RoPE.

Problem: b=4, n=2048, d=1024, H=16 heads, dk=64, rotary on first 32 dims
(interleaved pairs, theta=10000, lucidrains convention).

Sharding: 8 cores = 4 batches x 2 query-halves (data parallel). Each core
computes the full K/V projections for its batch (replicated across the 2
query-half siblings) and attention + output projection for its 1024 query
rows. No collectives needed; host gathers/concatenates.

Device-side layout strategy (all "transposed", features on partitions):
  - Host passes X.T (d-major) so projections need no on-device transposes.
  - Q.T/K.T [feat, rows] come straight out of the projection matmuls.
  - RoPE: host permutes Wq/Wk output features per head to [evens(16),
    pass(16), odds(16), pass(16)] so the interleaved pair rotation becomes
    a +-32 partition-offset multiply-add against cos/sin tables (dot
    products of q,k are invariant to a shared permutation). The +-32
    offsets keep every SBUF access pattern quadrant-aligned (SBUF APs may
    only start at partitions 0/32/64/96).
  - scores.T = K_rot.T' @ Q_rot.T per head (contraction=64 feats); the two
    heads of a 128-partition chunk run as concurrent row-group matmuls.
  - softmax: scores are O(1) so exp without max-subtraction; the
    normalizer Z comes free from a ones-column appended to V; attention
    output is produced transposed (Y.T) so the output projection needs no
    transposes either; host transposes the final [d, rows] shard.
"""

import os

import numpy as np

B, N, D = 4, 2048, 1024
H, DK = 16, 64
ROT, HALF = 32, 16
THETA = 10000.0
NCORES = 8
NQ = N // 2  # query rows per core

_PROGRAM_CACHE = {}


def _build_program(mm_dtype_name="bfloat16"):
    import concourse.tile as tile
    from concourse import bacc, mybir
    from contextlib import ExitStack

    PHASES = int(os.environ.get("KPHASES", "9"))  # debug bisect knob
    KLOOP = int(os.environ.get("KLOOP", "1"))      # hw-loop repeat (timing)

    f32 = mybir.dt.float32
    mmdt = getattr(mybir.dt, mm_dtype_name)
    AF = mybir.ActivationFunctionType
    ALU = mybir.AluOpType

    nc = bacc.Bacc("TRN2", target_bir_lowering=False)

    # DRAM I/O (per core). All *T tensors are feature-major (transposed).
    xqT = nc.dram_tensor("xqT", [D, NQ], mmdt, kind="ExternalInput")
    xkT = nc.dram_tensor("xkT", [D, N], mmdt, kind="ExternalInput")
    xvT = nc.dram_tensor("xvT", [D, N], mmdt, kind="ExternalInput")
    wqT = nc.dram_tensor("wqT", [D, D], mmdt, kind="ExternalInput")
    wkT = nc.dram_tensor("wkT", [D, D], mmdt, kind="ExternalInput")
    wvT = nc.dram_tensor("wvT", [D, H * 65], mmdt, kind="ExternalInput")
    wvb = nc.dram_tensor("wvb", [1, H * 65], mmdt, kind="ExternalInput")
    woT = nc.dram_tensor("woT", [D, D], mmdt, kind="ExternalInput")
    bq_d = nc.dram_tensor("bq_d", [D], f32, kind="ExternalInput")
    bk_d = nc.dram_tensor("bk_d", [D], f32, kind="ExternalInput")
    bqs_d = nc.dram_tensor("bqs_d", [D], f32, kind="ExternalInput")
    bks_d = nc.dram_tensor("bks_d", [D], f32, kind="ExternalInput")
    bo_d = nc.dram_tensor("bo_d", [D], f32, kind="ExternalInput")
    cosQ = nc.dram_tensor("cosQ", [128, NQ], mmdt, kind="ExternalInput")
    sinQ = nc.dram_tensor("sinQ", [128, NQ], mmdt, kind="ExternalInput")
    cosK = nc.dram_tensor("cosK", [128, N], mmdt, kind="ExternalInput")
    sinK = nc.dram_tensor("sinK", [128, N], mmdt, kind="ExternalInput")
    outT = nc.dram_tensor("outT", [D, NQ], f32, kind="ExternalOutput")

    NKC = N // 128       # 16 key chunks
    NFC = D // 128       # 8 feature chunks
    NVC = D // 128       # 8 contraction chunks for V

    with ExitStack() as ctx:
        tc = ctx.enter_context(tile.TileContext(nc))

        const = ctx.enter_context(tc.tile_pool(name="const", bufs=1))
        dram = ctx.enter_context(tc.tile_pool(name="dram", bufs=2, space="DRAM"))

        # persistent sbuf tensors
        v_sb = const.tile([128, NKC, H * 65], mmdt)   # V' (keys, per-head 64+ones)
        q_sb = const.tile([128, NFC, NQ], mmdt)       # Q_rot.T
        k_sb = const.tile([128, NFC, N], mmdt)        # K_rot.T
        y_sb = const.tile([128, NFC, NQ], mmdt)       # Y.T (normalized attn out)
        bq_sb = const.tile([128, NFC], f32)
        bk_sb = const.tile([128, NFC], f32)
        bqs_sb = const.tile([128, NFC], f32)
        bks_sb = const.tile([128, NFC], f32)
        bo_sb = const.tile([128, NFC], f32)
        cq_sb = const.tile([128, NQ], mmdt)
        sq_sb = const.tile([128, NQ], mmdt)
        ck_sb = const.tile([128, N], mmdt)
        sk_sb = const.tile([128, N], mmdt)
        ones1 = const.tile([1, 128], mmdt)
        wvb_sb = const.tile([1, H * 65], mmdt)
        wq_sb = const.tile([128, NFC, D], mmdt)
        nc.vector.memset(ones1[:], 1.0)
        nc.sync.dma_start(wvb_sb[:], wvb[:])

        _dmaq = [nc.sync, nc.scalar, nc.gpsimd]
        _dmaqi = [0]

        def dma_rr(dst, src_ap):
            eng = _dmaq[_dmaqi[0] % len(_dmaq)]
            _dmaqi[0] += 1
            eng.dma_start(dst, src_ap)

        def load_chunked(dst_tile, src_t, nchunks, splits=4):
            # dst [128, nchunks, cols]; src (c p) cols layout
            per = nchunks // splits if nchunks % splits == 0 else 1
            if per == 0:
                per = 1
            c = 0
            while c < nchunks:
                n = min(per, nchunks - c)
                dma_rr(
                    dst_tile[:, c:c + n, :],
                    src_t[c * 128:(c + n) * 128, :].rearrange(
                        "(c p) r -> p c r", p=128),
                )
                c += n

        nc.sync.dma_start(bq_sb[:], bq_d.rearrange("(c p) -> p c", p=128))
        nc.sync.dma_start(bk_sb[:], bk_d.rearrange("(c p) -> p c", p=128))
        nc.sync.dma_start(bqs_sb[:], bqs_d.rearrange("(c p) -> p c", p=128))
        nc.sync.dma_start(bks_sb[:], bks_d.rearrange("(c p) -> p c", p=128))
        nc.sync.dma_start(bo_sb[:], bo_d.rearrange("(c p) -> p c", p=128))
        load_chunked(wq_sb, wqT, NFC)
        nc.sync.dma_start(cq_sb[:], cosQ[:])
        nc.sync.dma_start(sq_sb[:], sinQ[:])
        nc.sync.dma_start(ck_sb[:], cosK[:])
        nc.sync.dma_start(sk_sb[:], sinK[:])

        def phase_v():
            with tc.tile_pool(name="vphase", bufs=1) as vp, \
                 tc.tile_pool(name="vpsum", bufs=4, space="PSUM") as vps:
                xv_sb = vp.tile([128, NVC, N], mmdt)
                wv_sb = vp.tile([128, NVC, H * 65], mmdt)
                load_chunked(xv_sb, xvT, NVC)
                load_chunked(wv_sb, wvT, NVC)
                for kc in range(NKC):
                    for nf in range(4):  # 1040 = 4 * 260
                        ps = vps.tile([128, 260], f32, tag="vps")
                        for dc in range(NVC):
                            nc.tensor.matmul(
                                ps[:],
                                lhsT=xv_sb[:, dc, kc * 128:(kc + 1) * 128],
                                rhs=wv_sb[:, dc, nf * 260:(nf + 1) * 260],
                                start=(dc == 0),
                                stop=False,
                            )
                        # bias + ones row (K=1): V' gets +bv and the Z column
                        nc.tensor.matmul(
                            ps[:],
                            lhsT=ones1[:, kc % 1:kc % 1 + 128],
                            rhs=wvb_sb[:, nf * 260:(nf + 1) * 260],
                            start=False,
                            stop=True,
                        )
                        nc.scalar.activation(
                            v_sb[:, kc, nf * 260:(nf + 1) * 260], ps[:], AF.Identity
                        )

        # Persistent zeroed sin-term temporaries: pass rows stay zero forever;
        # only the 4x16 rotary rows are rewritten each block.
        tmpS_tiles = [
            const.tile([128, 1024], f32, tag=f"tmpS{i}", name=f"tmpS{i}")
            for i in (0, 1)
        ]
        for t in tmpS_tiles:
            nc.vector.memset(t[:], 0.0)

        # Per-head feature layout (after the host permutation):
        #   [0:16) evens, [16:32) pass, [32:48) odds, [48:64) pass
        # so rotary partners are at +-32 partitions (quadrant aligned).
        def proj_rope(x_sb, w_sb, b_sb, bs_sb, cos_sb, sin_sb, dst_sb, nrows,
                      rope_pool, rope_psum):
            # dst.T[feat_chunk] over row blocks of 1024
            for fc in range(NFC):
                for rb in range(nrows // 1024):
                    r0 = rb * 1024
                    ps = rope_psum.tile([128, 1024], f32, tag="qk_ps")
                    for dc in range(NFC):
                        for h512 in range(2):
                            nc.tensor.matmul(
                                ps[:, h512 * 512:(h512 + 1) * 512],
                                lhsT=w_sb[:, dc, fc * 128:(fc + 1) * 128],
                                rhs=x_sb[:, dc, r0 + h512 * 512:r0 + (h512 + 1) * 512],
                                start=(dc == 0),
                                stop=(dc == NFC - 1),
                            )
                    tmpC = rope_pool.tile([128, 1024], f32, tag="tmpC")
                    tmpS = tmpS_tiles[(fc + rb) % 2]
                    # cos part (bias folded): tmpC = (ps + b) * cos
                    nc.vector.scalar_tensor_tensor(
                        tmpC[:], ps[:], b_sb[:, fc:fc + 1],
                        cos_sb[:, r0:r0 + 1024], op0=ALU.add, op1=ALU.mult,
                    )
                    # sin part: partner rows at +-32, sign folded into sin table
                    for h2 in (0, 64):
                        nc.vector.scalar_tensor_tensor(
                            tmpS[h2:h2 + 16, :],
                            ps[h2 + 32:h2 + 48, :],
                            bs_sb[h2:h2 + 16, fc:fc + 1],
                            sin_sb[h2:h2 + 16, r0:r0 + 1024],
                            op0=ALU.add, op1=ALU.mult,
                        )
                        nc.vector.scalar_tensor_tensor(
                            tmpS[h2 + 32:h2 + 48, :],
                            ps[h2:h2 + 16, :],
                            bs_sb[h2 + 32:h2 + 48, fc:fc + 1],
                            sin_sb[h2 + 32:h2 + 48, r0:r0 + 1024],
                            op0=ALU.add, op1=ALU.mult,
                        )
                    nc.vector.tensor_add(
                        dst_sb[:, fc, r0:r0 + 1024], tmpC[:], tmpS[:]
                    )

        def phase_q(wk_sb):
            with tc.tile_pool(name="qphase", bufs=1) as qp, \
                 tc.tile_pool(name="qrope", bufs=3) as qrp, \
                 tc.tile_pool(name="qpsum", bufs=2, space="PSUM") as qps:
                xq_sb = qp.tile([128, NFC, NQ], mmdt)
                load_chunked(xq_sb, xqT, NFC)
                load_chunked(wk_sb, wkT, NFC)  # prefetch K weights
                proj_rope(xq_sb, wq_sb, bq_sb, bqs_sb, cq_sb, sq_sb, q_sb, NQ,
                          qrp, qps)

        def phase_k(wk_sb):
            with tc.tile_pool(name="kphase", bufs=1) as kp, \
                 tc.tile_pool(name="krope", bufs=3) as krp, \
                 tc.tile_pool(name="kpsum", bufs=2, space="PSUM") as kps:
                xk_sb = kp.tile([128, NFC, N], mmdt)
                load_chunked(xk_sb, xkT, NFC)
                proj_rope(xk_sb, wk_sb, bk_sb, bks_sb, ck_sb, sk_sb, k_sb, N,
                          krp, kps)

        def phase_attn():
            with tc.tile_pool(name="spsum", bufs=2, space="PSUM") as sps, \
                 tc.tile_pool(name="opsum", bufs=2, space="PSUM") as ops_pool, \
                 tc.tile_pool(name="ppool", bufs=4) as pp, \
                 tc.tile_pool(name="npool", bufs=2) as npl:
                for h in range(H):
                    fc = h // 2
                    hb = (h % 2) * 64
                    po = ops_pool.tile([65, NQ], f32, tag="po")
                    for kc in range(NKC):
                        ps = sps.tile([128, NQ], f32, tag="st")
                        for qn in range(NQ // 512):
                            nc.tensor.matmul(
                                ps[:, qn * 512:(qn + 1) * 512],
                                lhsT=k_sb[hb:hb + 64, fc, kc * 128:(kc + 1) * 128],
                                rhs=q_sb[hb:hb + 64, fc, qn * 512:(qn + 1) * 512],
                                start=True,
                                stop=True,
                            )
                        pt = pp.tile([128, NQ], mmdt, tag="pt")
                        nc.scalar.activation(pt[:], ps[:], AF.Exp, scale=1.0 / 8.0)
                        for qn in range(NQ // 512):
                            nc.tensor.matmul(
                                po[:, qn * 512:(qn + 1) * 512],
                                lhsT=v_sb[:, kc, h * 65:(h + 1) * 65],
                                rhs=pt[:, qn * 512:(qn + 1) * 512],
                                start=(kc == 0),
                                stop=(kc == NKC - 1),
                            )
                    # normalize: y.T[head rows] = po[0:64] * (1/Z) broadcast
                    rz = npl.tile([1, NQ], mmdt, tag="rz")
                    with nc.allow_low_precision(reason="1/Z in bf16 matches bf16 P/V noise"):
                        nc.vector.reciprocal(rz[:], po[64:65, :])
                    rz_dram = dram.tile([1, NQ], mmdt, tag="rzd")
                    nc.sync.dma_start(rz_dram[:], rz[:])
                    rzb = npl.tile([64, NQ], mmdt, tag="rzb")
                    nc.sync.dma_start(rzb[:], rz_dram[:].to_broadcast([64, NQ]))
                    nc.vector.tensor_mul(
                        y_sb[hb:hb + 64, fc, :], po[0:64, :], rzb[:]
                    )

        def phase_out():
            with tc.tile_pool(name="ophase", bufs=1) as op_pool, \
                 tc.tile_pool(name="owork", bufs=3) as owork, \
                 tc.tile_pool(name="opsum2", bufs=4, space="PSUM") as ops2:
                wo_sb = op_pool.tile([128, NFC, D], mmdt)
                load_chunked(wo_sb, woT, NFC)
                for dmc in range(NFC):
                    ob = owork.tile([128, NQ], f32, tag="ob")
                    for rn in range(NQ // 512):
                        ps = ops2.tile([128, 512], f32, tag="ops")
                        for fc in range(NFC):
                            nc.tensor.matmul(
                                ps[:],
                                lhsT=wo_sb[:, fc, dmc * 128:(dmc + 1) * 128],
                                rhs=y_sb[:, fc, rn * 512:(rn + 1) * 512],
                                start=(fc == 0),
                                stop=(fc == NFC - 1),
                            )
                        nc.vector.tensor_scalar_add(
                            ob[:, rn * 512:(rn + 1) * 512], ps[:],
                            bo_sb[:, dmc:dmc + 1])
                    eng = nc.sync if dmc % 2 == 0 else nc.gpsimd
                    eng.dma_start(outT[dmc * 128:(dmc + 1) * 128, :], ob[:])

        def all_phases():
            if PHASES >= 1:
                phase_v()
            with tc.tile_pool(name="kw", bufs=1) as kwp:
                wk_sb = kwp.tile([128, NFC, D], mmdt)
                if PHASES >= 2:
                    phase_q(wk_sb)
                if PHASES >= 3:
                    phase_k(wk_sb)
            if PHASES >= 4:
                phase_attn()
            else:
                nc.vector.memset(y_sb[:], 0.0)
            if PHASES >= 5:
                phase_out()
            else:
                with tc.tile_pool(name="dummy", bufs=1) as dp:
                    zb = dp.tile([128, NQ], f32)
                    nc.vector.memset(zb[:], 0.0)
                    for dmc in range(NFC):
                        nc.sync.dma_start(outT[dmc * 128:(dmc + 1) * 128, :], zb[:])

        if KLOOP > 1:
            with tc.For_i(0, KLOOP, 1):
                all_phases()
        else:
            all_phases()

    nc.compile()
    return nc


def _rope_tables(positions):
    """cos/sin tables [128, len(positions)] for the permuted transposed
    layout: partition p (within a 2-head feature chunk), j = p % 64:
    j<16: freq j (cos, -sin); 32<=j<48: freq j-32 (cos, +sin); else (1, 0)."""
    inv_freq = 1.0 / (THETA ** (np.arange(0, ROT, 2, dtype=np.float64) / ROT))  # [16]
    t = np.asarray(positions, dtype=np.float64)
    ang = t[None, :] * inv_freq[:, None]  # [16, nt]
    c, s = np.cos(ang), np.sin(ang)
    cos_tab = np.ones((128, len(positions)), dtype=np.float64)
    sin_tab = np.zeros((128, len(positions)), dtype=np.float64)
    for h2 in (0, 64):
        cos_tab[h2:h2 + 16] = c
        cos_tab[h2 + 32:h2 + 48] = c
        sin_tab[h2:h2 + 16] = -s
        sin_tab[h2 + 32:h2 + 48] = s
    return cos_tab.astype(np.float32), sin_tab.astype(np.float32)


def _head_perm():
    """Feature permutation applied to rows of Wq/Wk (and bq/bk): within each
    head's 64 outputs -> [evens(16), pass 32:48, odds(16), pass 48:64]."""
    out = np.empty(D, dtype=np.int64)
    for h in range(H):
        base = h * DK
        out[base:base + HALF] = base + np.arange(0, ROT, 2)
        out[base + HALF:base + ROT] = base + np.arange(ROT, ROT + HALF)
        out[base + ROT:base + ROT + HALF] = base + np.arange(1, ROT, 2)
        out[base + ROT + HALF:base + DK] = base + np.arange(ROT + HALF, DK)
    return out


def _partner_map():
    """Index map m with m[p] = rotary partner of permuted feature p
    (p XOR 32 within a 64-feature head for rot rows; identity for pass)."""
    m = np.arange(D, dtype=np.int64)
    for h in range(H):
        base = h * DK
        m[base:base + HALF] = base + ROT + np.arange(HALF)
        m[base + ROT:base + ROT + HALF] = base + np.arange(HALF)
    return m


def _prep_inputs(query, key, value, Wq, bq, Wk, bk, Wv, bv, Wo, bo,
                 mm_dtype_name="bfloat16"):
    import ml_dtypes

    np_mm = ml_dtypes.bfloat16 if mm_dtype_name == "bfloat16" else np.float32

    query = np.asarray(query, np.float32)
    key = np.asarray(key, np.float32)
    value = np.asarray(value, np.float32)
    Wq, bq = np.asarray(Wq, np.float32), np.asarray(bq, np.float32)
    Wk, bk = np.asarray(Wk, np.float32), np.asarray(bk, np.float32)
    Wv, bv = np.asarray(Wv, np.float32), np.asarray(bv, np.float32)
    Wo, bo = np.asarray(Wo, np.float32), np.asarray(bo, np.float32)

    perm = _head_perm()
    Wq_p, bq_p = Wq[perm], bq[perm]
    Wk_p, bk_p = Wk[perm], bk[perm]
    swp = _partner_map()
    bq_s, bk_s = bq_p[swp], bk_p[swp]

    wqT = np.ascontiguousarray(Wq_p.T).astype(np_mm)
    wkT = np.ascontiguousarray(Wk_p.T).astype(np_mm)
    woT = np.ascontiguousarray(Wo.T).astype(np_mm)

    # W_v' : [D, H*65] plus a separate bias/ones row wvb [1, H*65]
    wvT = np.zeros((D, H * 65), np.float32)
    wvb = np.zeros((1, H * 65), np.float32)
    for h in range(H):
        cols = slice(h * 65, h * 65 + 64)
        wvT[:D, cols] = Wv[h * DK:(h + 1) * DK, :].T
        wvb[0, cols] = bv[h * DK:(h + 1) * DK]
        wvb[0, h * 65 + 64] = 1.0
    wvT = wvT.astype(np_mm)
    wvb = wvb.astype(np_mm)

    cos_all, sin_all = _rope_tables(np.arange(N))

    in_maps = []
    for core in range(NCORES):
        b, qh = core // 2, core % 2
        rows = slice(qh * NQ, (qh + 1) * NQ)
        xqT = np.ascontiguousarray(query[b, rows, :].T).astype(np_mm)
        xkT = np.ascontiguousarray(key[b].T).astype(np_mm)
        xvT = np.ascontiguousarray(value[b].T).astype(np_mm)
        in_maps.append({
            "xqT": xqT,
            "xkT": xkT,
            "xvT": xvT,
            "wqT": wqT, "wkT": wkT, "wvT": wvT, "woT": woT, "wvb": wvb,
            "bq_d": bq_p, "bk_d": bk_p, "bo_d": bo,
            "bqs_d": bq_s, "bks_d": bk_s,
            "cosQ": np.ascontiguousarray(cos_all[:, rows]).astype(np_mm),
            "sinQ": np.ascontiguousarray(sin_all[:, rows]).astype(np_mm),
            "cosK": cos_all.astype(np_mm),
            "sinK": sin_all.astype(np_mm),
        })
    return in_maps


def kernel(query, key, value, Wq, bq, Wk, bk, Wv, bv, Wo, bo):
    from concourse import bass_utils

    mm_dtype_name = "bfloat16"
    if mm_dtype_name not in _PROGRAM_CACHE:
        _PROGRAM_CACHE[mm_dtype_name] = _build_program(mm_dtype_name)
    nc = _PROGRAM_CACHE[mm_dtype_name]

    in_maps = _prep_inputs(query, key, value, Wq, bq, Wk, bk, Wv, bv, Wo, bo,
                           mm_dtype_name)

    res = bass_utils.run_bass_kernel_spmd(
        nc, in_maps, core_ids=list(range(NCORES))
    )

    out = np.empty((B, N, D), np.float32)
    for core in range(NCORES):
        b, qh = core // 2, core % 2
        out[b, qh * NQ:(qh + 1) * NQ, :] = res.results[core]["outT"].T
    return out


# revision 18
# speedup vs baseline: 1.0014x; 1.0014x over previous
"""Trainium2 Bass kernel for MultiHeadedAttention with RoPE.

Problem: b=4, n=2048, d=1024, H=16 heads, dk=64, rotary on first 32 dims
(interleaved pairs, theta=10000, lucidrains convention).

Sharding: 8 cores = 4 batches x 2 query-halves (data parallel). Each core
computes the full K/V projections for its batch (replicated across the 2
query-half siblings) and attention + output projection for its 1024 query
rows. No collectives needed; host gathers/concatenates.

Device-side layout strategy (all "transposed", features on partitions):
  - Host passes X.T (d-major) so projections need no on-device transposes.
  - Q.T/K.T [feat, rows] come straight out of the projection matmuls.
  - RoPE: host permutes Wq/Wk output features per head to [evens(16),
    pass(16), odds(16), pass(16)] so the interleaved pair rotation becomes
    a +-32 partition-offset multiply-add against cos/sin tables (dot
    products of q,k are invariant to a shared permutation). The +-32
    offsets keep every SBUF access pattern quadrant-aligned (SBUF APs may
    only start at partitions 0/32/64/96).
  - scores.T = K_rot.T' @ Q_rot.T per head (contraction=64 feats); the two
    heads of a 128-partition chunk run as concurrent row-group matmuls.
  - softmax: scores are O(1) so exp without max-subtraction; the
    normalizer Z comes free from a ones-column appended to V; attention
    output is produced transposed (Y.T) so the output projection needs no
    transposes either; host transposes the final [d, rows] shard.
"""

import os

import numpy as np

B, N, D = 4, 2048, 1024
H, DK = 16, 64
ROT, HALF = 32, 16
THETA = 10000.0
NCORES = 8
NQ = N // 2  # query rows per core

_PROGRAM_CACHE = {}


def _build_program(mm_dtype_name="bfloat16"):
    import concourse.tile as tile
    from concourse import bacc, mybir
    from contextlib import ExitStack

    PHASES = int(os.environ.get("KPHASES", "9"))  # debug bisect knob
    KLOOP = int(os.environ.get("KLOOP", "1"))      # hw-loop repeat (timing)

    f32 = mybir.dt.float32
    mmdt = getattr(mybir.dt, mm_dtype_name)
    AF = mybir.ActivationFunctionType
    ALU = mybir.AluOpType

    nc = bacc.Bacc("TRN2", target_bir_lowering=False)

    # DRAM I/O (per core). All *T tensors are feature-major (transposed).
    xqT = nc.dram_tensor("xqT", [D, NQ], mmdt, kind="ExternalInput")
    xkT = nc.dram_tensor("xkT", [D, N], mmdt, kind="ExternalInput")
    xvT = nc.dram_tensor("xvT", [D, N], mmdt, kind="ExternalInput")
    wqT = nc.dram_tensor("wqT", [D, D], mmdt, kind="ExternalInput")
    wkT = nc.dram_tensor("wkT", [D, D], mmdt, kind="ExternalInput")
    wvT = nc.dram_tensor("wvT", [D, H * 65], mmdt, kind="ExternalInput")
    wvb = nc.dram_tensor("wvb", [1, H * 65], mmdt, kind="ExternalInput")
    woT = nc.dram_tensor("woT", [D, D], mmdt, kind="ExternalInput")
    bq_d = nc.dram_tensor("bq_d", [D], f32, kind="ExternalInput")
    bk_d = nc.dram_tensor("bk_d", [D], f32, kind="ExternalInput")
    bqs_d = nc.dram_tensor("bqs_d", [D], f32, kind="ExternalInput")
    bks_d = nc.dram_tensor("bks_d", [D], f32, kind="ExternalInput")
    bo_d = nc.dram_tensor("bo_d", [D], f32, kind="ExternalInput")
    cosQ = nc.dram_tensor("cosQ", [128, NQ], mmdt, kind="ExternalInput")
    sinQ = nc.dram_tensor("sinQ", [128, NQ], mmdt, kind="ExternalInput")
    cosK = nc.dram_tensor("cosK", [128, N], mmdt, kind="ExternalInput")
    sinK = nc.dram_tensor("sinK", [128, N], mmdt, kind="ExternalInput")
    outT = nc.dram_tensor("outT", [D, NQ], f32, kind="ExternalOutput")

    NKC = N // 128       # 16 key chunks
    NFC = D // 128       # 8 feature chunks
    NVC = D // 128       # 8 contraction chunks for V

    with ExitStack() as ctx:
        tc = ctx.enter_context(tile.TileContext(nc))

        const = ctx.enter_context(tc.tile_pool(name="const", bufs=1))
        dram = ctx.enter_context(tc.tile_pool(name="dram", bufs=2, space="DRAM"))

        # persistent sbuf tensors
        v_sb = const.tile([128, NKC, H * 65], mmdt)   # V' (keys, per-head 64+ones)
        q_sb = const.tile([128, NFC, NQ], mmdt)       # Q_rot.T
        k_sb = const.tile([128, NFC, N], mmdt)        # K_rot.T
        y_sb = const.tile([128, NFC, NQ], mmdt)       # Y.T (normalized attn out)
        bq_sb = const.tile([128, NFC], f32)
        bk_sb = const.tile([128, NFC], f32)
        bqs_sb = const.tile([128, NFC], f32)
        bks_sb = const.tile([128, NFC], f32)
        bo_sb = const.tile([128, NFC], f32)
        cq_sb = const.tile([128, NQ], mmdt)
        sq_sb = const.tile([128, NQ], mmdt)
        ck_sb = const.tile([128, N], mmdt)
        sk_sb = const.tile([128, N], mmdt)
        ones1 = const.tile([1, 128], mmdt)
        wvb_sb = const.tile([1, H * 65], mmdt)
        wq_sb = const.tile([128, NFC, D], mmdt)
        nc.vector.memset(ones1[:], 1.0)
        nc.sync.dma_start(wvb_sb[:], wvb[:])

        _dmaq = [nc.sync, nc.scalar, nc.gpsimd]
        _dmaqi = [0]

        def dma_rr(dst, src_ap):
            eng = _dmaq[_dmaqi[0] % len(_dmaq)]
            _dmaqi[0] += 1
            eng.dma_start(dst, src_ap)

        def load_chunked(dst_tile, src_t, nchunks, splits=4):
            # dst [128, nchunks, cols]; src (c p) cols layout
            per = nchunks // splits if nchunks % splits == 0 else 1
            if per == 0:
                per = 1
            c = 0
            while c < nchunks:
                n = min(per, nchunks - c)
                dma_rr(
                    dst_tile[:, c:c + n, :],
                    src_t[c * 128:(c + n) * 128, :].rearrange(
                        "(c p) r -> p c r", p=128),
                )
                c += n

        nc.sync.dma_start(bq_sb[:], bq_d.rearrange("(c p) -> p c", p=128))
        nc.sync.dma_start(bk_sb[:], bk_d.rearrange("(c p) -> p c", p=128))
        nc.sync.dma_start(bqs_sb[:], bqs_d.rearrange("(c p) -> p c", p=128))
        nc.sync.dma_start(bks_sb[:], bks_d.rearrange("(c p) -> p c", p=128))
        nc.sync.dma_start(bo_sb[:], bo_d.rearrange("(c p) -> p c", p=128))
        nc.sync.dma_start(cq_sb[:], cosQ[:])
        nc.sync.dma_start(sq_sb[:], sinQ[:])
        nc.sync.dma_start(ck_sb[:], cosK[:])
        nc.sync.dma_start(sk_sb[:], sinK[:])

        def phase_v():
            with tc.tile_pool(name="vphase", bufs=1) as vp, \
                 tc.tile_pool(name="vpsum", bufs=4, space="PSUM") as vps:
                xv_sb = vp.tile([128, NVC, N], mmdt)
                wv_sb = vp.tile([128, NVC, H * 65], mmdt)
                load_chunked(xv_sb, xvT, NVC)
                load_chunked(wv_sb, wvT, NVC)
                load_chunked(wq_sb, wqT, NFC)  # prefetch Q weights behind V loads
                for kc in range(NKC):
                    for nf in range(4):  # 1040 = 4 * 260
                        ps = vps.tile([128, 260], f32, tag="vps")
                        for dc in range(NVC):
                            nc.tensor.matmul(
                                ps[:],
                                lhsT=xv_sb[:, dc, kc * 128:(kc + 1) * 128],
                                rhs=wv_sb[:, dc, nf * 260:(nf + 1) * 260],
                                start=(dc == 0),
                                stop=False,
                            )
                        # bias + ones row (K=1): V' gets +bv and the Z column
                        nc.tensor.matmul(
                            ps[:],
                            lhsT=ones1[:, kc % 1:kc % 1 + 128],
                            rhs=wvb_sb[:, nf * 260:(nf + 1) * 260],
                            start=False,
                            stop=True,
                        )
                        nc.scalar.activation(
                            v_sb[:, kc, nf * 260:(nf + 1) * 260], ps[:], AF.Identity
                        )

        # Persistent zeroed sin-term temporaries: pass rows stay zero forever;
        # only the 4x16 rotary rows are rewritten each block.
        tmpS_tiles = [
            const.tile([128, 1024], f32, tag=f"tmpS{i}", name=f"tmpS{i}")
            for i in (0, 1)
        ]
        for t in tmpS_tiles:
            nc.vector.memset(t[:], 0.0)

        # Per-head feature layout (after the host permutation):
        #   [0:16) evens, [16:32) pass, [32:48) odds, [48:64) pass
        # so rotary partners are at +-32 partitions (quadrant aligned).
        def proj_rope(x_sb, w_sb, b_sb, bs_sb, cos_sb, sin_sb, dst_sb, nrows,
                      rope_pool, rope_psum):
            # dst.T[feat_chunk] over row blocks of 1024
            for fc in range(NFC):
                for rb in range(nrows // 1024):
                    r0 = rb * 1024
                    ps = rope_psum.tile([128, 1024], f32, tag="qk_ps")
                    for dc in range(NFC):
                        for h512 in range(2):
                            nc.tensor.matmul(
                                ps[:, h512 * 512:(h512 + 1) * 512],
                                lhsT=w_sb[:, dc, fc * 128:(fc + 1) * 128],
                                rhs=x_sb[:, dc, r0 + h512 * 512:r0 + (h512 + 1) * 512],
                                start=(dc == 0),
                                stop=(dc == NFC - 1),
                            )
                    tmpC = rope_pool.tile([128, 1024], f32, tag="tmpC")
                    tmpS = tmpS_tiles[(fc + rb) % 2]
                    # cos part (bias folded): tmpC = (ps + b) * cos
                    nc.vector.scalar_tensor_tensor(
                        tmpC[:], ps[:], b_sb[:, fc:fc + 1],
                        cos_sb[:, r0:r0 + 1024], op0=ALU.add, op1=ALU.mult,
                    )
                    # sin part: partner rows at +-32, sign folded into sin table
                    for h2 in (0, 64):
                        nc.vector.scalar_tensor_tensor(
                            tmpS[h2:h2 + 16, :],
                            ps[h2 + 32:h2 + 48, :],
                            bs_sb[h2:h2 + 16, fc:fc + 1],
                            sin_sb[h2:h2 + 16, r0:r0 + 1024],
                            op0=ALU.add, op1=ALU.mult,
                        )
                        nc.vector.scalar_tensor_tensor(
                            tmpS[h2 + 32:h2 + 48, :],
                            ps[h2:h2 + 16, :],
                            bs_sb[h2 + 32:h2 + 48, fc:fc + 1],
                            sin_sb[h2 + 32:h2 + 48, r0:r0 + 1024],
                            op0=ALU.add, op1=ALU.mult,
                        )
                    nc.vector.tensor_add(
                        dst_sb[:, fc, r0:r0 + 1024], tmpC[:], tmpS[:]
                    )

        def phase_q(wk_sb):
            with tc.tile_pool(name="qphase", bufs=1) as qp, \
                 tc.tile_pool(name="qrope", bufs=3) as qrp, \
                 tc.tile_pool(name="qpsum", bufs=2, space="PSUM") as qps:
                xq_sb = qp.tile([128, NFC, NQ], mmdt)
                load_chunked(xq_sb, xqT, NFC)
                load_chunked(wk_sb, wkT, NFC)  # prefetch K weights
                proj_rope(xq_sb, wq_sb, bq_sb, bqs_sb, cq_sb, sq_sb, q_sb, NQ,
                          qrp, qps)

        def phase_k(wk_sb):
            with tc.tile_pool(name="kphase", bufs=1) as kp, \
                 tc.tile_pool(name="krope", bufs=3) as krp, \
                 tc.tile_pool(name="kpsum", bufs=2, space="PSUM") as kps:
                xk_sb = kp.tile([128, NFC, N], mmdt)
                load_chunked(xk_sb, xkT, NFC)
                proj_rope(xk_sb, wk_sb, bk_sb, bks_sb, ck_sb, sk_sb, k_sb, N,
                          krp, kps)

        def phase_attn():
            with tc.tile_pool(name="spsum", bufs=2, space="PSUM") as sps, \
                 tc.tile_pool(name="opsum", bufs=2, space="PSUM") as ops_pool, \
                 tc.tile_pool(name="ppool", bufs=4) as pp, \
                 tc.tile_pool(name="npool", bufs=2) as npl:
                for h in range(H):
                    fc = h // 2
                    hb = (h % 2) * 64
                    po = ops_pool.tile([65, NQ], f32, tag="po")
                    for kc in range(NKC):
                        ps = sps.tile([128, NQ], f32, tag="st")
                        for qn in range(NQ // 512):
                            nc.tensor.matmul(
                                ps[:, qn * 512:(qn + 1) * 512],
                                lhsT=k_sb[hb:hb + 64, fc, kc * 128:(kc + 1) * 128],
                                rhs=q_sb[hb:hb + 64, fc, qn * 512:(qn + 1) * 512],
                                start=True,
                                stop=True,
                            )
                        pt = pp.tile([128, NQ], mmdt, tag="pt")
                        nc.scalar.activation(pt[:], ps[:], AF.Exp, scale=1.0 / 8.0)
                        for qn in range(NQ // 512):
                            nc.tensor.matmul(
                                po[:, qn * 512:(qn + 1) * 512],
                                lhsT=v_sb[:, kc, h * 65:(h + 1) * 65],
                                rhs=pt[:, qn * 512:(qn + 1) * 512],
                                start=(kc == 0),
                                stop=(kc == NKC - 1),
                            )
                    # normalize: y.T[head rows] = po[0:64] * (1/Z) broadcast
                    rz = npl.tile([1, NQ], mmdt, tag="rz")
                    with nc.allow_low_precision(reason="1/Z in bf16 matches bf16 P/V noise"):
                        nc.vector.reciprocal(rz[:], po[64:65, :])
                    rz_dram = dram.tile([1, NQ], mmdt, tag="rzd")
                    nc.sync.dma_start(rz_dram[:], rz[:])
                    rzb = npl.tile([64, NQ], mmdt, tag="rzb")
                    nc.sync.dma_start(rzb[:], rz_dram[:].to_broadcast([64, NQ]))
                    nc.vector.tensor_mul(
                        y_sb[hb:hb + 64, fc, :], po[0:64, :], rzb[:]
                    )

        def phase_out():
            with tc.tile_pool(name="ophase", bufs=1) as op_pool, \
                 tc.tile_pool(name="owork", bufs=3) as owork, \
                 tc.tile_pool(name="opsum2", bufs=4, space="PSUM") as ops2:
                wo_sb = op_pool.tile([128, NFC, D], mmdt)
                load_chunked(wo_sb, woT, NFC)
                for dmc in range(NFC):
                    ob = owork.tile([128, NQ], f32, tag="ob")
                    for rn in range(NQ // 512):
                        ps = ops2.tile([128, 512], f32, tag="ops")
                        for fc in range(NFC):
                            nc.tensor.matmul(
                                ps[:],
                                lhsT=wo_sb[:, fc, dmc * 128:(dmc + 1) * 128],
                                rhs=y_sb[:, fc, rn * 512:(rn + 1) * 512],
                                start=(fc == 0),
                                stop=(fc == NFC - 1),
                            )
                        nc.vector.tensor_scalar_add(
                            ob[:, rn * 512:(rn + 1) * 512], ps[:],
                            bo_sb[:, dmc:dmc + 1])
                    eng = nc.sync if dmc % 2 == 0 else nc.gpsimd
                    eng.dma_start(outT[dmc * 128:(dmc + 1) * 128, :], ob[:])

        def all_phases():
            if PHASES >= 1:
                phase_v()
            with tc.tile_pool(name="kw", bufs=1) as kwp:
                wk_sb = kwp.tile([128, NFC, D], mmdt)
                if PHASES >= 2:
                    phase_q(wk_sb)
                if PHASES >= 3:
                    phase_k(wk_sb)
            if PHASES >= 4:
                phase_attn()
            else:
                nc.vector.memset(y_sb[:], 0.0)
            if PHASES >= 5:
                phase_out()
            else:
                with tc.tile_pool(name="dummy", bufs=1) as dp:
                    zb = dp.tile([128, NQ], f32)
                    nc.vector.memset(zb[:], 0.0)
                    for dmc in range(NFC):
                        nc.sync.dma_start(outT[dmc * 128:(dmc + 1) * 128, :], zb[:])

        if KLOOP > 1:
            with tc.For_i(0, KLOOP, 1):
                all_phases()
        else:
            all_phases()

    nc.compile()
    return nc


def _rope_tables(positions):
    """cos/sin tables [128, len(positions)] for the permuted transposed
    layout: partition p (within a 2-head feature chunk), j = p % 64:
    j<16: freq j (cos, -sin); 32<=j<48: freq j-32 (cos, +sin); else (1, 0)."""
    inv_freq = 1.0 / (THETA ** (np.arange(0, ROT, 2, dtype=np.float64) / ROT))  # [16]
    t = np.asarray(positions, dtype=np.float64)
    ang = t[None, :] * inv_freq[:, None]  # [16, nt]
    c, s = np.cos(ang), np.sin(ang)
    cos_tab = np.ones((128, len(positions)), dtype=np.float64)
    sin_tab = np.zeros((128, len(positions)), dtype=np.float64)
    for h2 in (0, 64):
        cos_tab[h2:h2 + 16] = c
        cos_tab[h2 + 32:h2 + 48] = c
        sin_tab[h2:h2 + 16] = -s
        sin_tab[h2 + 32:h2 + 48] = s
    return cos_tab.astype(np.float32), sin_tab.astype(np.float32)


def _head_perm():
    """Feature permutation applied to rows of Wq/Wk (and bq/bk): within each
    head's 64 outputs -> [evens(16), pass 32:48, odds(16), pass 48:64]."""
    out = np.empty(D, dtype=np.int64)
    for h in range(H):
        base = h * DK
        out[base:base + HALF] = base + np.arange(0, ROT, 2)
        out[base + HALF:base + ROT] = base + np.arange(ROT, ROT + HALF)
        out[base + ROT:base + ROT + HALF] = base + np.arange(1, ROT, 2)
        out[base + ROT + HALF:base + DK] = base + np.arange(ROT + HALF, DK)
    return out


def _partner_map():
    """Index map m with m[p] = rotary partner of permuted feature p
    (p XOR 32 within a 64-feature head for rot rows; identity for pass)."""
    m = np.arange(D, dtype=np.int64)
    for h in range(H):
        base = h * DK
        m[base:base + HALF] = base + ROT + np.arange(HALF)
        m[base + ROT:base + ROT + HALF] = base + np.arange(HALF)
    return m


def _prep_inputs(query, key, value, Wq, bq, Wk, bk, Wv, bv, Wo, bo,
                 mm_dtype_name="bfloat16"):
    import ml_dtypes

    np_mm = ml_dtypes.bfloat16 if mm_dtype_name == "bfloat16" else np.float32

    query = np.asarray(query, np.float32)
    key = np.asarray(key, np.float32)
    value = np.asarray(value, np.float32)
    Wq, bq = np.asarray(Wq, np.float32), np.asarray(bq, np.float32)
    Wk, bk = np.asarray(Wk, np.float32), np.asarray(bk, np.float32)
    Wv, bv = np.asarray(Wv, np.float32), np.asarray(bv, np.float32)
    Wo, bo = np.asarray(Wo, np.float32), np.asarray(bo, np.float32)

    perm = _head_perm()
    Wq_p, bq_p = Wq[perm], bq[perm]
    Wk_p, bk_p = Wk[perm], bk[perm]
    swp = _partner_map()
    bq_s, bk_s = bq_p[swp], bk_p[swp]

    wqT = np.ascontiguousarray(Wq_p.T).astype(np_mm)
    wkT = np.ascontiguousarray(Wk_p.T).astype(np_mm)
    woT = np.ascontiguousarray(Wo.T).astype(np_mm)

    # W_v' : [D, H*65] plus a separate bias/ones row wvb [1, H*65]
    wvT = np.zeros((D, H * 65), np.float32)
    wvb = np.zeros((1, H * 65), np.float32)
    for h in range(H):
        cols = slice(h * 65, h * 65 + 64)
        wvT[:D, cols] = Wv[h * DK:(h + 1) * DK, :].T
        wvb[0, cols] = bv[h * DK:(h + 1) * DK]
        wvb[0, h * 65 + 64] = 1.0
    wvT = wvT.astype(np_mm)
    wvb = wvb.astype(np_mm)

    cos_all, sin_all = _rope_tables(np.arange(N))

    in_maps = []
    for core in range(NCORES):
        b, qh = core // 2, core % 2
        rows = slice(qh * NQ, (qh + 1) * NQ)
        xqT = np.ascontiguousarray(query[b, rows, :].T).astype(np_mm)
        xkT = np.ascontiguousarray(key[b].T).astype(np_mm)
        xvT = np.ascontiguousarray(value[b].T).astype(np_mm)
        in_maps.append({
            "xqT": xqT,
            "xkT": xkT,
            "xvT": xvT,
            "wqT": wqT, "wkT": wkT, "wvT": wvT, "woT": woT, "wvb": wvb,
            "bq_d": bq_p, "bk_d": bk_p, "bo_d": bo,
            "bqs_d": bq_s, "bks_d": bk_s,
            "cosQ": np.ascontiguousarray(cos_all[:, rows]).astype(np_mm),
            "sinQ": np.ascontiguousarray(sin_all[:, rows]).astype(np_mm),
            "cosK": cos_all.astype(np_mm),
            "sinK": sin_all.astype(np_mm),
        })
    return in_maps


def kernel(query, key, value, Wq, bq, Wk, bk, Wv, bv, Wo, bo):
    from concourse import bass_utils

    mm_dtype_name = "bfloat16"
    if mm_dtype_name not in _PROGRAM_CACHE:
        _PROGRAM_CACHE[mm_dtype_name] = _build_program(mm_dtype_name)
    nc = _PROGRAM_CACHE[mm_dtype_name]

    in_maps = _prep_inputs(query, key, value, Wq, bq, Wk, bk, Wv, bv, Wo, bo,
                           mm_dtype_name)

    res = bass_utils.run_bass_kernel_spmd(
        nc, in_maps, core_ids=list(range(NCORES))
    )

    out = np.empty((B, N, D), np.float32)
    for core in range(NCORES):
        b, qh = core // 2, core % 2
        out[b, qh * NQ:(qh + 1) * NQ, :] = res.results[core]["outT"].T
    return out


# revision 20
# speedup vs baseline: 1.1692x; 1.1675x over previous
"""Trainium2 Bass kernel for MultiHeadedAttention with RoPE.

Problem: b=4, n=2048, d=1024, H=16 heads, dk=64, rotary on first 32 dims
(interleaved pairs, theta=10000, lucidrains convention).

Sharding: 8 cores = 4 batches x 2 query-halves (data parallel). Each core
computes the full K/V projections for its batch (replicated across the 2
query-half siblings) and attention + output projection for its 1024 query
rows. No collectives needed; host gathers/concatenates.

Device-side layout strategy (all "transposed", features on partitions):
  - Host passes X.T (d-major) so projections need no on-device transposes.
  - Q.T/K.T [feat, rows] come straight out of the projection matmuls.
  - RoPE: host permutes Wq/Wk output features per head to [evens(16),
    pass(16), odds(16), pass(16)] so the interleaved pair rotation becomes
    a +-32 partition-offset multiply-add against cos/sin tables (dot
    products of q,k are invariant to a shared permutation). The +-32
    offsets keep every SBUF access pattern quadrant-aligned (SBUF APs may
    only start at partitions 0/32/64/96).
  - scores.T = K_rot.T' @ Q_rot.T per head (contraction=64 feats); the two
    heads of a 128-partition chunk run as concurrent row-group matmuls.
  - softmax: scores are O(1) so exp without max-subtraction; the
    normalizer Z comes free from a ones-column appended to V; attention
    output is produced transposed (Y.T) so the output projection needs no
    transposes either; host transposes the final [d, rows] shard.
"""

import os

import numpy as np

B, N, D = 4, 2048, 1024
H, DK = 16, 64
ROT, HALF = 32, 16
THETA = 10000.0
NCORES = 8
NQ = N // 2  # query rows per core

_PROGRAM_CACHE = {}


def _build_program(mm_dtype_name="bfloat16"):
    import concourse.tile as tile
    from concourse import bacc, mybir
    from contextlib import ExitStack

    PHASES = int(os.environ.get("KPHASES", "9"))  # debug bisect knob
    KLOOP = int(os.environ.get("KLOOP", "1"))      # hw-loop repeat (timing)

    f32 = mybir.dt.float32
    mmdt = getattr(mybir.dt, mm_dtype_name)
    AF = mybir.ActivationFunctionType
    ALU = mybir.AluOpType

    nc = bacc.Bacc("TRN2", target_bir_lowering=False)

    # DRAM I/O (per core). All *T tensors are feature-major (transposed).
    xqT = nc.dram_tensor("xqT", [D, NQ], mmdt, kind="ExternalInput")
    xkT = nc.dram_tensor("xkT", [D, N], mmdt, kind="ExternalInput")
    xvT = nc.dram_tensor("xvT", [D, N], mmdt, kind="ExternalInput")
    wqT = nc.dram_tensor("wqT", [D, D], mmdt, kind="ExternalInput")
    wkT = nc.dram_tensor("wkT", [D, D], mmdt, kind="ExternalInput")
    wvT = nc.dram_tensor("wvT", [D, H * 65], mmdt, kind="ExternalInput")
    wvb = nc.dram_tensor("wvb", [1, H * 65], mmdt, kind="ExternalInput")
    woT = nc.dram_tensor("woT", [D, D], mmdt, kind="ExternalInput")
    bq_d = nc.dram_tensor("bq_d", [D], f32, kind="ExternalInput")
    bk_d = nc.dram_tensor("bk_d", [D], f32, kind="ExternalInput")
    bqs_d = nc.dram_tensor("bqs_d", [D], f32, kind="ExternalInput")
    bks_d = nc.dram_tensor("bks_d", [D], f32, kind="ExternalInput")
    bo_d = nc.dram_tensor("bo_d", [D], f32, kind="ExternalInput")
    cosQ = nc.dram_tensor("cosQ", [128, NQ], mmdt, kind="ExternalInput")
    sinQ = nc.dram_tensor("sinQ", [128, NQ], mmdt, kind="ExternalInput")
    cosK = nc.dram_tensor("cosK", [128, N], mmdt, kind="ExternalInput")
    sinK = nc.dram_tensor("sinK", [128, N], mmdt, kind="ExternalInput")
    outT = nc.dram_tensor("outT", [D, NQ], f32, kind="ExternalOutput")

    NKC = N // 128       # 16 key chunks
    NFC = D // 128       # 8 feature chunks
    NVC = D // 128       # 8 contraction chunks for V

    with ExitStack() as ctx:
        tc = ctx.enter_context(tile.TileContext(nc))

        const = ctx.enter_context(tc.tile_pool(name="const", bufs=1))
        dram = ctx.enter_context(tc.tile_pool(name="dram", bufs=2, space="DRAM"))

        # persistent sbuf tensors
        v_sb = const.tile([128, NKC, H * 65], mmdt)   # V' (keys, per-head 64+ones)
        q_sb = const.tile([128, NFC, NQ], mmdt)       # Q_rot.T
        k_sb = const.tile([128, NFC, N], mmdt)        # K_rot.T
        y_sb = const.tile([128, NFC, NQ], mmdt)       # Y.T (normalized attn out)
        bq_sb = const.tile([128, NFC], f32)
        bk_sb = const.tile([128, NFC], f32)
        bqs_sb = const.tile([128, NFC], f32)
        bks_sb = const.tile([128, NFC], f32)
        bo_sb = const.tile([128, NFC], f32)
        cq_sb = const.tile([128, NQ], mmdt)
        sq_sb = const.tile([128, NQ], mmdt)
        ck_sb = const.tile([128, N], mmdt)
        sk_sb = const.tile([128, N], mmdt)
        ones1 = const.tile([1, 128], mmdt)
        wvb_sb = const.tile([1, H * 65], mmdt)
        nc.vector.memset(ones1[:], 1.0)
        nc.sync.dma_start(wvb_sb[:], wvb[:])

        _dmaq = [nc.sync, nc.scalar, nc.gpsimd]
        _dmaqi = [0]

        def dma_rr(dst, src_ap):
            eng = _dmaq[_dmaqi[0] % len(_dmaq)]
            _dmaqi[0] += 1
            eng.dma_start(dst, src_ap)

        def load_chunked(dst_tile, src_t, nchunks, splits=4):
            # dst [128, nchunks, cols]; src (c p) cols layout
            per = nchunks // splits if nchunks % splits == 0 else 1
            if per == 0:
                per = 1
            c = 0
            while c < nchunks:
                n = min(per, nchunks - c)
                dma_rr(
                    dst_tile[:, c:c + n, :],
                    src_t[c * 128:(c + n) * 128, :].rearrange(
                        "(c p) r -> p c r", p=128),
                )
                c += n

        nc.sync.dma_start(bq_sb[:], bq_d.rearrange("(c p) -> p c", p=128))
        nc.sync.dma_start(bk_sb[:], bk_d.rearrange("(c p) -> p c", p=128))
        nc.sync.dma_start(bqs_sb[:], bqs_d.rearrange("(c p) -> p c", p=128))
        nc.sync.dma_start(bks_sb[:], bks_d.rearrange("(c p) -> p c", p=128))
        nc.sync.dma_start(bo_sb[:], bo_d.rearrange("(c p) -> p c", p=128))
        nc.sync.dma_start(cq_sb[:], cosQ[:])
        nc.sync.dma_start(sq_sb[:], sinQ[:])
        nc.sync.dma_start(ck_sb[:], cosK[:])
        nc.sync.dma_start(sk_sb[:], sinK[:])

        def phase_v():
            with tc.tile_pool(name="vphase", bufs=1) as vp, \
                 tc.tile_pool(name="vpsum", bufs=4, space="PSUM") as vps:
                xv_sb = vp.tile([128, NVC, N], mmdt)
                wv_sb = vp.tile([128, NVC, H * 65], mmdt)
                load_chunked(xv_sb, xvT, NVC)
                load_chunked(wv_sb, wvT, NVC)
                for kc in range(NKC):
                    for nf in range(4):  # 1040 = 4 * 260
                        ps = vps.tile([128, 260], f32, tag="vps")
                        for dc in range(NVC):
                            nc.tensor.matmul(
                                ps[:],
                                lhsT=xv_sb[:, dc, kc * 128:(kc + 1) * 128],
                                rhs=wv_sb[:, dc, nf * 260:(nf + 1) * 260],
                                start=(dc == 0),
                                stop=False,
                            )
                        # bias + ones row (K=1): V' gets +bv and the Z column
                        nc.tensor.matmul(
                            ps[:],
                            lhsT=ones1[:, kc % 1:kc % 1 + 128],
                            rhs=wvb_sb[:, nf * 260:(nf + 1) * 260],
                            start=False,
                            stop=True,
                        )
                        nc.scalar.activation(
                            v_sb[:, kc, nf * 260:(nf + 1) * 260], ps[:], AF.Identity
                        )

        # Persistent zeroed sin-term temporaries: pass rows stay zero forever;
        # only the 4x16 rotary rows are rewritten each block.
        tmpS_tiles = [
            const.tile([128, 1024], f32, tag=f"tmpS{i}", name=f"tmpS{i}")
            for i in (0, 1)
        ]
        for t in tmpS_tiles:
            nc.vector.memset(t[:], 0.0)

        # Per-head feature layout (after the host permutation):
        #   [0:16) evens, [16:32) pass, [32:48) odds, [48:64) pass
        # so rotary partners are at +-32 partitions (quadrant aligned).
        def proj_rope(x_sb, w_sb, b_sb, bs_sb, cos_sb, sin_sb, dst_sb, nrows,
                      rope_pool, rope_psum):
            # dst.T[feat_chunk] over row blocks of 1024
            for fc in range(NFC):
                for rb in range(nrows // 1024):
                    r0 = rb * 1024
                    ps = rope_psum.tile([128, 1024], f32, tag="qk_ps")
                    for dc in range(NFC):
                        for h512 in range(2):
                            nc.tensor.matmul(
                                ps[:, h512 * 512:(h512 + 1) * 512],
                                lhsT=w_sb[:, dc, fc * 128:(fc + 1) * 128],
                                rhs=x_sb[:, dc, r0 + h512 * 512:r0 + (h512 + 1) * 512],
                                start=(dc == 0),
                                stop=(dc == NFC - 1),
                            )
                    tmpC = rope_pool.tile([128, 1024], f32, tag="tmpC")
                    tmpS = tmpS_tiles[(fc + rb) % 2]
                    # cos part (bias folded): tmpC = (ps + b) * cos
                    nc.vector.scalar_tensor_tensor(
                        tmpC[:], ps[:], b_sb[:, fc:fc + 1],
                        cos_sb[:, r0:r0 + 1024], op0=ALU.add, op1=ALU.mult,
                    )
                    # sin part: partner rows at +-32, sign folded into sin table
                    for h2 in (0, 64):
                        nc.vector.scalar_tensor_tensor(
                            tmpS[h2:h2 + 16, :],
                            ps[h2 + 32:h2 + 48, :],
                            bs_sb[h2:h2 + 16, fc:fc + 1],
                            sin_sb[h2:h2 + 16, r0:r0 + 1024],
                            op0=ALU.add, op1=ALU.mult,
                        )
                        nc.vector.scalar_tensor_tensor(
                            tmpS[h2 + 32:h2 + 48, :],
                            ps[h2:h2 + 16, :],
                            bs_sb[h2 + 32:h2 + 48, fc:fc + 1],
                            sin_sb[h2 + 32:h2 + 48, r0:r0 + 1024],
                            op0=ALU.add, op1=ALU.mult,
                        )
                    nc.vector.tensor_add(
                        dst_sb[:, fc, r0:r0 + 1024], tmpC[:], tmpS[:]
                    )

        def phase_q():
            with tc.tile_pool(name="qphase", bufs=1) as qp, \
                 tc.tile_pool(name="qrope", bufs=3) as qrp, \
                 tc.tile_pool(name="qpsum", bufs=3, space="PSUM") as qps:
                xq_sb = qp.tile([128, NFC, NQ], mmdt)
                wq_sb = qp.tile([128, NFC, D], mmdt)
                load_chunked(xq_sb, xqT, NFC)
                load_chunked(wq_sb, wqT, NFC)
                proj_rope(xq_sb, wq_sb, bq_sb, bqs_sb, cq_sb, sq_sb, q_sb, NQ,
                          qrp, qps)

        def phase_k():
            with tc.tile_pool(name="kphase", bufs=1) as kp, \
                 tc.tile_pool(name="krope", bufs=3) as krp, \
                 tc.tile_pool(name="kpsum", bufs=3, space="PSUM") as kps:
                xk_sb = kp.tile([128, NFC, N], mmdt)
                wk_sb = kp.tile([128, NFC, D], mmdt)
                load_chunked(xk_sb, xkT, NFC)
                load_chunked(wk_sb, wkT, NFC)
                proj_rope(xk_sb, wk_sb, bk_sb, bks_sb, ck_sb, sk_sb, k_sb, N,
                          krp, kps)

        def phase_attn():
            with tc.tile_pool(name="spsum", bufs=2, space="PSUM") as sps, \
                 tc.tile_pool(name="opsum", bufs=2, space="PSUM") as ops_pool, \
                 tc.tile_pool(name="ppool", bufs=3) as pp, \
                 tc.tile_pool(name="npool", bufs=2) as npl:
                for h in range(H):
                    fc = h // 2
                    hb = (h % 2) * 64
                    po = ops_pool.tile([65, NQ], f32, tag="po")
                    for kc in range(NKC):
                        ps = sps.tile([128, NQ], f32, tag="st")
                        for qn in range(NQ // 512):
                            nc.tensor.matmul(
                                ps[:, qn * 512:(qn + 1) * 512],
                                lhsT=k_sb[hb:hb + 64, fc, kc * 128:(kc + 1) * 128],
                                rhs=q_sb[hb:hb + 64, fc, qn * 512:(qn + 1) * 512],
                                start=True,
                                stop=True,
                            )
                        pt = pp.tile([128, NQ], mmdt, tag="pt")
                        nc.scalar.activation(pt[:], ps[:], AF.Exp, scale=1.0 / 8.0)
                        for qn in range(NQ // 512):
                            nc.tensor.matmul(
                                po[:, qn * 512:(qn + 1) * 512],
                                lhsT=v_sb[:, kc, h * 65:(h + 1) * 65],
                                rhs=pt[:, qn * 512:(qn + 1) * 512],
                                start=(kc == 0),
                                stop=(kc == NKC - 1),
                            )
                    # normalize: y.T[head rows] = po[0:64] * (1/Z) broadcast
                    rz = npl.tile([1, NQ], f32, tag="rz")
                    nc.vector.reciprocal(rz[:], po[64:65, :])
                    rz_dram = dram.tile([1, NQ], f32, tag="rzd")
                    nc.sync.dma_start(rz_dram[:], rz[:])
                    rzb = npl.tile([64, NQ], f32, tag="rzb")
                    nc.sync.dma_start(rzb[:], rz_dram[:].to_broadcast([64, NQ]))
                    nc.vector.tensor_mul(
                        y_sb[hb:hb + 64, fc, :], po[0:64, :], rzb[:]
                    )

        def phase_out():
            with tc.tile_pool(name="ophase", bufs=1) as op_pool, \
                 tc.tile_pool(name="owork", bufs=3) as owork, \
                 tc.tile_pool(name="opsum2", bufs=4, space="PSUM") as ops2:
                wo_sb = op_pool.tile([128, NFC, D], mmdt)
                load_chunked(wo_sb, woT, NFC)
                for dmc in range(NFC):
                    ob = owork.tile([128, NQ], f32, tag="ob")
                    for rn in range(NQ // 512):
                        ps = ops2.tile([128, 512], f32, tag="ops")
                        for fc in range(NFC):
                            nc.tensor.matmul(
                                ps[:],
                                lhsT=wo_sb[:, fc, dmc * 128:(dmc + 1) * 128],
                                rhs=y_sb[:, fc, rn * 512:(rn + 1) * 512],
                                start=(fc == 0),
                                stop=(fc == NFC - 1),
                            )
                        nc.vector.tensor_scalar_add(
                            ob[:, rn * 512:(rn + 1) * 512], ps[:],
                            bo_sb[:, dmc:dmc + 1])
                    eng = nc.sync if dmc % 2 == 0 else nc.gpsimd
                    eng.dma_start(outT[dmc * 128:(dmc + 1) * 128, :], ob[:])

        def all_phases():
            if PHASES >= 1:
                phase_v()
            if PHASES >= 2:
                phase_q()
            if PHASES >= 3:
                phase_k()
            if PHASES >= 4:
                phase_attn()
            else:
                nc.vector.memset(y_sb[:], 0.0)
            if PHASES >= 5:
                phase_out()
            else:
                with tc.tile_pool(name="dummy", bufs=1) as dp:
                    zb = dp.tile([128, NQ], f32)
                    nc.vector.memset(zb[:], 0.0)
                    for dmc in range(NFC):
                        nc.sync.dma_start(outT[dmc * 128:(dmc + 1) * 128, :], zb[:])

        if KLOOP > 1:
            with tc.For_i(0, KLOOP, 1):
                all_phases()
        else:
            all_phases()

    nc.compile()
    return nc


def _rope_tables(positions):
    """cos/sin tables [128, len(positions)] for the permuted transposed
    layout: partition p (within a 2-head feature chunk), j = p % 64:
    j<16: freq j (cos, -sin); 32<=j<48: freq j-32 (cos, +sin); else (1, 0)."""
    inv_freq = 1.0 / (THETA ** (np.arange(0, ROT, 2, dtype=np.float64) / ROT))  # [16]
    t = np.asarray(positions, dtype=np.float64)
    ang = t[None, :] * inv_freq[:, None]  # [16, nt]
    c, s = np.cos(ang), np.sin(ang)
    cos_tab = np.ones((128, len(positions)), dtype=np.float64)
    sin_tab = np.zeros((128, len(positions)), dtype=np.float64)
    for h2 in (0, 64):
        cos_tab[h2:h2 + 16] = c
        cos_tab[h2 + 32:h2 + 48] = c
        sin_tab[h2:h2 + 16] = -s
        sin_tab[h2 + 32:h2 + 48] = s
    return cos_tab.astype(np.float32), sin_tab.astype(np.float32)


def _head_perm():
    """Feature permutation applied to rows of Wq/Wk (and bq/bk): within each
    head's 64 outputs -> [evens(16), pass 32:48, odds(16), pass 48:64]."""
    out = np.empty(D, dtype=np.int64)
    for h in range(H):
        base = h * DK
        out[base:base + HALF] = base + np.arange(0, ROT, 2)
        out[base + HALF:base + ROT] = base + np.arange(ROT, ROT + HALF)
        out[base + ROT:base + ROT + HALF] = base + np.arange(1, ROT, 2)
        out[base + ROT + HALF:base + DK] = base + np.arange(ROT + HALF, DK)
    return out


def _partner_map():
    """Index map m with m[p] = rotary partner of permuted feature p
    (p XOR 32 within a 64-feature head for rot rows; identity for pass)."""
    m = np.arange(D, dtype=np.int64)
    for h in range(H):
        base = h * DK
        m[base:base + HALF] = base + ROT + np.arange(HALF)
        m[base + ROT:base + ROT + HALF] = base + np.arange(HALF)
    return m


def _prep_inputs(query, key, value, Wq, bq, Wk, bk, Wv, bv, Wo, bo,
                 mm_dtype_name="bfloat16"):
    import ml_dtypes

    np_mm = ml_dtypes.bfloat16 if mm_dtype_name == "bfloat16" else np.float32

    query = np.asarray(query, np.float32)
    key = np.asarray(key, np.float32)
    value = np.asarray(value, np.float32)
    Wq, bq = np.asarray(Wq, np.float32), np.asarray(bq, np.float32)
    Wk, bk = np.asarray(Wk, np.float32), np.asarray(bk, np.float32)
    Wv, bv = np.asarray(Wv, np.float32), np.asarray(bv, np.float32)
    Wo, bo = np.asarray(Wo, np.float32), np.asarray(bo, np.float32)

    perm = _head_perm()
    Wq_p, bq_p = Wq[perm], bq[perm]
    Wk_p, bk_p = Wk[perm], bk[perm]
    swp = _partner_map()
    bq_s, bk_s = bq_p[swp], bk_p[swp]

    wqT = np.ascontiguousarray(Wq_p.T).astype(np_mm)
    wkT = np.ascontiguousarray(Wk_p.T).astype(np_mm)
    woT = np.ascontiguousarray(Wo.T).astype(np_mm)

    # W_v' : [D, H*65] plus a separate bias/ones row wvb [1, H*65]
    wvT = np.zeros((D, H * 65), np.float32)
    wvb = np.zeros((1, H * 65), np.float32)
    for h in range(H):
        cols = slice(h * 65, h * 65 + 64)
        wvT[:D, cols] = Wv[h * DK:(h + 1) * DK, :].T
        wvb[0, cols] = bv[h * DK:(h + 1) * DK]
        wvb[0, h * 65 + 64] = 1.0
    wvT = wvT.astype(np_mm)
    wvb = wvb.astype(np_mm)

    cos_all, sin_all = _rope_tables(np.arange(N))

    in_maps = []
    for core in range(NCORES):
        b, qh = core // 2, core % 2
        rows = slice(qh * NQ, (qh + 1) * NQ)
        xqT = np.ascontiguousarray(query[b, rows, :].T).astype(np_mm)
        xkT = np.ascontiguousarray(key[b].T).astype(np_mm)
        xvT = np.ascontiguousarray(value[b].T).astype(np_mm)
        in_maps.append({
            "xqT": xqT,
            "xkT": xkT,
            "xvT": xvT,
            "wqT": wqT, "wkT": wkT, "wvT": wvT, "woT": woT, "wvb": wvb,
            "bq_d": bq_p, "bk_d": bk_p, "bo_d": bo,
            "bqs_d": bq_s, "bks_d": bk_s,
            "cosQ": np.ascontiguousarray(cos_all[:, rows]).astype(np_mm),
            "sinQ": np.ascontiguousarray(sin_all[:, rows]).astype(np_mm),
            "cosK": cos_all.astype(np_mm),
            "sinK": sin_all.astype(np_mm),
        })
    return in_maps


def kernel(query, key, value, Wq, bq, Wk, bk, Wv, bv, Wo, bo):
    from concourse import bass_utils

    mm_dtype_name = "bfloat16"
    if mm_dtype_name not in _PROGRAM_CACHE:
        _PROGRAM_CACHE[mm_dtype_name] = _build_program(mm_dtype_name)
    nc = _PROGRAM_CACHE[mm_dtype_name]

    in_maps = _prep_inputs(query, key, value, Wq, bq, Wk, bk, Wv, bv, Wo, bo,
                           mm_dtype_name)

    res = bass_utils.run_bass_kernel_spmd(
        nc, in_maps, core_ids=list(range(NCORES))
    )

    out = np.empty((B, N, D), np.float32)
    for core in range(NCORES):
        b, qh = core // 2, core % 2
        out[b, qh * NQ:(qh + 1) * NQ, :] = res.results[core]["outT"].T
    return out
